# revision 3
# baseline (speedup 1.0000x reference)
"""EIF neuron kernel for Trainium2 (Bass/Tile), 8-core data-parallel.

Reference semantics (TAU=1.0, V_TH=1.0, DELTA_T=0.2, V_RESET=0.0):
    e      = 0.2 * exp((m - 1) / 0.2)
    m'     = m + (x_t - m + e) / 1.0   == x_t + e
    spike  = (m' >= 1)
    m      = where(spike, 0, m')

The whole step runs on the VECTOR ENGINE ONLY as two chained custom-DVE
instructions (no ACT, no cross-engine semaphores in the serial chain):

    e == 0.2*e^{5(m-1)} == f(m)^16,  f(m) = 2^((K*m + AC)/16),
    K = 5/ln2, AC = log2(0.2) - K.  On the occupied domain m in (-6, 1),
    f is approximated by a monic quartic in v = m+1 (relative error 1.6e-6,
    e-term rel err ~2.5e-5), factored into two quadratics so the 8-stage
    DVE datapath fits it:

    INSTR1 (EIF_POLY_ANT):  v = m+1;  q = ((v+a)v+b) * ((v+g)v+d)
    INSTR2 (EIF_STEP_ANT):  e = ((q^2)^2 * a4^4)^2)^2   [= (a4*q)^16]
                            m' = e + x;   out = (m' < 1) * m'

    The quartic has no real roots and stays in [0.17, 0.49] for v<=0, so
    e <= 9e-6 there -- it clamps itself; no explicit clamp stage.

out doubles as the stored state for the next step AND the DMA'd result;
the host recovers spikes as bits(out) == +0.0 (the reset writes +0.0;
a no-spike m' is +-nonzero except for exact-zero sums, which are
measure-zero in fp32).  Measured fidelity vs the fp32 jax reference:
~20 flipped spikes of 67M (budget at rel_err 2e-2 is ~4300).

Sharding: batch dim B=32 -> 4 batches per core; per core the (b,n) set
is 128 partitions x 128 free columns per timestep; T=512 serial steps,
2 DVE instructions each, DMA in/out in 64-step chunks, double-buffered.
"""

import numpy as np
from contextlib import ExitStack

import concourse.bass as bass
import concourse.bacc as bacc
import concourse.tile as tile
from concourse import mybir
from concourse.bass_utils import run_bass_kernel_spmd

F32 = mybir.dt.float32

B, T, N = 32, 512, 4096
NCORES = 8
BPC = B // NCORES            # 4 batches per core
P = 128                      # SBUF partitions
FD = (BPC * N) // P          # 128 free columns per timestep
TC = 64                      # timesteps per DMA chunk

# Quartic fit of f(v) = 2^((K*(v-1) + AC)/16) over v in [0, 2] (v = m+1),
# monic product form p/a4 = (v^2 + AL*v + BE)(v^2 + GA*v + DE); fitted with
# iteratively-reweighted relative LS on Chebyshev nodes (fit_poly2.py).
A4 = np.float32(0.0002618293444520486)
AL = np.float32(9.165841509136218)
BE = np.float32(28.991836577868735)
GA = np.float32(-0.23563442502622411)
DE = np.float32(63.76567319169257)
LAM = np.float32(np.float32(A4 * A4) * np.float32(A4 * A4))   # a4^4

_registered = None
_built = None


def _f32(x):
    return np.asarray(x, np.float32)


def _ref_poly(in0, in1, s0, s1, imm2):
    """out = ((v+s0)v+s1) * ((v+imm2)v+d), v = in0+1, d = in1[P,1] (C3)."""
    f32 = np.float32
    d = np.asarray(in1, f32).reshape(in0.shape[0], -1)[:, :1]
    v = _f32(in0.astype(f32) + f32(1.0))
    q1 = _f32(_f32(_f32(v + f32(s0)) * v) + f32(s1))
    q2 = _f32(_f32(_f32(v + f32(imm2)) * v) + d)
    return _f32(q1 * q2)


def _ref_step(in0, in1, s0, s1, imm2):
    """out = (m'<1)*m', m' = ((in0^2)^2 * s1)^2)^2 + in1 (elementwise)."""
    f32 = np.float32
    q = in0.astype(f32)
    x = np.asarray(in1, f32).reshape(q.shape[0], -1)
    t0 = _f32(q * q)
    t1 = _f32(t0 * t0)
    t1l = _f32(t1 * f32(s1))
    t2 = _f32(t1l * t1l)
    e = _f32(t2 * t2)
    mp = _f32(e + x.reshape(q.shape))
    return _f32((mp < f32(1.0)).astype(f32) * mp)


def _register_ops():
    """Register the two EIF custom-DVE ops in concourse.dve_ops (the
    documented extension point -- appended, never reordered)."""
    global _registered
    if _registered is not None:
        return _registered
    from concourse import dve_ops
    from concourse.dve_spec import (
        Spec, Src0, Src1, C0, C1, C2, C3, One, lower, _spill_c3_to_src1,
        _has_src1,
    )
    from concourse.dve_uop import DveOpSpec

    v = Src0 + One
    q1 = (v + C0) * v + C1
    q2 = (v + C2) * v + C3
    spec_poly = Spec(body=_spill_c3_to_src1(q1 * q2), reference=_ref_poly)

    Q = Src0
    t0 = Q * Q
    t1 = t0 * t0
    t1l = t1 * C1
    t2 = t1l * t1l
    e = t2 * t2
    mp = e + Src1
    spec_step = Spec(body=(mp < One) * mp, reference=_ref_step)

    ops = []
    for name, spec in (("EIF_POLY_ANT", spec_poly), ("EIF_STEP_ANT", spec_step)):
        existing = [o for o in dve_ops.OPS if o.name == name]
        if existing:
            ops.append(existing[0])
            continue
        row = dve_ops._CUSTOM_DVE_ROW_BASE + len(dve_ops.OPS)
        shas = {}
        for ver in ("v3", "v4"):
            try:
                uops = lower(spec, ver=ver)
            except Exception:
                continue
            shas[ver] = DveOpSpec(
                name=name, opcode=row, uops=uops, rd1_en=_has_src1(spec)
            ).sha(ver)
        op = dve_ops.DveOp(name, spec, subdim=False, uops_sha=shas)
        dve_ops.OPS.append(op)
        dve_ops.CUSTOM_DVE_SPECS[name] = spec
        dve_ops._SUB_OPCODE_FOR_NAME[name] = row
        assert row < 0x20
        ops.append(op)
    _registered = tuple(ops)
    return _registered


def _build(reps=1, tc=TC, qbufs=2, xbufs=2, sbufs=2):
    op_poly, op_step = _register_ops()
    nc = bacc.Bacc("TRN2", debug=False, num_devices=NCORES)
    x_d = nc.declare_dram_parameter("x", [P, T * FD], F32, isOutput=False)
    s_d = nc.declare_dram_parameter("spk", [P, T * FD], F32, isOutput=True)

    with ExitStack() as ctx:
        tcx = ctx.enter_context(tile.TileContext(nc))
        xpool = ctx.enter_context(tcx.tile_pool(name="xin", bufs=xbufs))
        spool = ctx.enter_context(tcx.tile_pool(name="sout", bufs=sbufs))
        state = ctx.enter_context(tcx.tile_pool(name="state", bufs=1))

        mz = state.tile([P, FD], F32, name="mz", tag="mz")
        dlt = state.tile([P, 1], F32, name="dlt", tag="dlt")
        nc.vector.memset(mz[:], 0.0)
        nc.vector.memset(dlt[:], float(DE))
        q = [state.tile([P, FD], F32, name=f"q{p}", tag=f"q{p}")
             for p in range(qbufs)]

        prev = mz  # [P, FD] AP holding the previous step's state
        prev_sp = None
        for _rep in range(reps):
            for ci in range(T // tc):
                xt = xpool.tile([P, tc * FD], F32, name="xt", tag="x")
                nc.sync.dma_start(
                    out=xt[:], in_=x_d[:, ci * tc * FD:(ci + 1) * tc * FD]
                )
                xt3 = xt.rearrange("p (t f) -> p t f", f=FD)
                sp = spool.tile([P, tc * FD], F32, name="sp", tag="s")
                sp3 = sp.rearrange("p (t f) -> p t f", f=FD)

                for k in range(tc):
                    qt = q[k % qbufs]
                    src = prev[:] if prev is not None else prev_sp
                    # INSTR1: q = quartic(m_prev + 1)
                    nc.vector._custom_dve(
                        op_poly, out=qt[:], in0=src, in1=dlt[:],
                        s0=float(AL), s1=float(BE), imm2=float(GA),
                    )
                    # INSTR2: out = (m'<1)*m', m' = (a4*q)^16 + x
                    nc.vector._custom_dve(
                        op_step, out=sp3[:, k, :], in0=qt[:],
                        in1=xt3[:, k:k + 1, :], s1=float(LAM),
                    )
                    prev, prev_sp = None, sp3[:, k, :]
                nc.sync.dma_start(
                    out=s_d[:, ci * tc * FD:(ci + 1) * tc * FD], in_=sp[:]
                )
    nc.compile()
    return nc


def _shard(x):
    """x[B,T,N] -> per-core [P, T*FD] partition-major arrays."""
    maps = []
    for c in range(NCORES):
        xc = x[c * BPC:(c + 1) * BPC]                      # [4, T, 4096]
        xc = np.ascontiguousarray(
            xc.reshape(BPC, T, N // FD, FD).transpose(0, 2, 1, 3)
        ).reshape(P, T * FD)
        maps.append({"x": xc})
    return maps


def _unshard(results):
    out = np.empty((B, T, N), np.float32)
    for c in range(NCORES):
        r = np.ascontiguousarray(np.asarray(results[c]["spk"]))
        r = r.reshape(BPC, N // FD, T, FD).transpose(0, 2, 1, 3).reshape(BPC, T, N)
        # spike fired iff the reset wrote +0.0 (bit-exact test)
        out[c * BPC:(c + 1) * BPC] = (r.view(np.uint32) == 0).astype(np.float32)
    return out


def kernel(x):
    global _built
    x = np.asarray(x, dtype=np.float32)
    assert x.shape == (B, T, N), x.shape
    if _built is None:
        _built = _build()
    res = run_bass_kernel_spmd(_built, _shard(x), list(range(NCORES)))
    return _unshard(res.results)


# revision 4
# speedup vs baseline: 1.5302x; 1.5302x over previous
"""EIF neuron kernel for Trainium2 (Bass/Tile), 8-core data-parallel.

Reference semantics (TAU=1.0, V_TH=1.0, DELTA_T=0.2, V_RESET=0.0):
    e      = 0.2 * exp((m - 1) / 0.2)
    m'     = m + (x_t - m + e) / 1.0   == x_t + e
    spike  = (m' >= 1)
    m      = where(spike, 0, m')

The whole step runs on the VECTOR ENGINE ONLY as two chained custom-DVE
instructions (no ACT, no cross-engine semaphores in the serial chain):

    e == 0.2*e^{5(m-1)} == f(m)^16,  f(m) = 2^((K*m + AC)/16),
    K = 5/ln2, AC = log2(0.2) - K.  On the occupied domain m in (-6, 1),
    f is approximated by a monic quartic in v = m+1 (relative error 1.6e-6,
    e-term rel err ~2.5e-5), factored into two quadratics so the 8-stage
    DVE datapath fits it:

    INSTR1 (EIF_POLY_ANT):  v = m+1;  q = ((v+a)v+b) * ((v+g)v+d)
    INSTR2 (EIF_STEP_ANT):  e = ((q^2)^2 * a4^4)^2)^2   [= (a4*q)^16]
                            m' = e + x;   out = (m' < 1) * m'

    The quartic has no real roots and stays in [0.17, 0.49] for v<=0, so
    e <= 9e-6 there -- it clamps itself; no explicit clamp stage.

out doubles as the stored state for the next step AND the DMA'd result;
the host recovers spikes as bits(out) == +0.0 (the reset writes +0.0;
a no-spike m' is +-nonzero except for exact-zero sums, which are
measure-zero in fp32).  Measured fidelity vs the fp32 jax reference:
~20 flipped spikes of 67M (budget at rel_err 2e-2 is ~4300).

Sharding: batch dim B=32 -> 4 batches per core; per core the (b,n) set
is 128 partitions x 128 free columns per timestep; T=512 serial steps,
2 DVE instructions each, DMA in/out in 64-step chunks, double-buffered.
"""

import numpy as np
from contextlib import ExitStack

import concourse.bass as bass
import concourse.bacc as bacc
import concourse.tile as tile
from concourse import mybir
from concourse.bass_utils import run_bass_kernel_spmd

F32 = mybir.dt.float32

B, T, N = 32, 512, 4096
NCORES = 8
BPC = B // NCORES            # 4 batches per core
P = 128                      # SBUF partitions
FD = (BPC * N) // P          # 128 free columns per timestep
TC = 64                      # timesteps per DMA chunk

# Quartic fit of f(v) = 2^((K*(v-1) + AC)/16) over v in [0, 2] (v = m+1),
# monic product form p/a4 = (v^2 + AL*v + BE)(v^2 + GA*v + DE); fitted with
# iteratively-reweighted relative LS on Chebyshev nodes (fit_poly2.py).
A4 = np.float32(0.0002618293444520486)
AL = np.float32(9.165841509136218)
BE = np.float32(28.991836577868735)
GA = np.float32(-0.23563442502622411)
DE = np.float32(63.76567319169257)
LAM = np.float32(np.float32(A4 * A4) * np.float32(A4 * A4))   # a4^4

_registered = None
_built = None


def _f32(x):
    return np.asarray(x, np.float32)


def _ref_poly(in0, in1, s0, s1, imm2):
    """out = ((v+s0)v+s1) * ((v+imm2)v+d), v = in0+1, d = in1[P,1] (C3)."""
    f32 = np.float32
    d = np.asarray(in1, f32).reshape(in0.shape[0], -1)[:, :1]
    v = _f32(in0.astype(f32) + f32(1.0))
    q1 = _f32(_f32(_f32(v + f32(s0)) * v) + f32(s1))
    q2 = _f32(_f32(_f32(v + f32(imm2)) * v) + d)
    return _f32(q1 * q2)


def _ref_step(in0, in1, s0, s1, imm2):
    """out = (m'<1)*m', m' = ((in0^2)^2 * s1)^2)^2 + in1 (elementwise)."""
    f32 = np.float32
    q = in0.astype(f32)
    x = np.asarray(in1, f32).reshape(q.shape[0], -1)
    t0 = _f32(q * q)
    t1 = _f32(t0 * t0)
    t1l = _f32(t1 * f32(s1))
    t2 = _f32(t1l * t1l)
    e = _f32(t2 * t2)
    mp = _f32(e + x.reshape(q.shape))
    return _f32((mp < f32(1.0)).astype(f32) * mp)


def _register_ops():
    """Register the two EIF custom-DVE ops in concourse.dve_ops (the
    documented extension point -- appended, never reordered)."""
    global _registered
    if _registered is not None:
        return _registered
    from concourse import dve_ops
    from concourse.dve_spec import (
        Spec, Src0, Src1, C0, C1, C2, C3, One, lower, _spill_c3_to_src1,
        _has_src1,
    )
    from concourse.dve_uop import DveOpSpec

    v = Src0 + One
    q1 = (v + C0) * v + C1
    q2 = (v + C2) * v + C3
    spec_poly = Spec(body=_spill_c3_to_src1(q1 * q2), reference=_ref_poly)

    Q = Src0
    t0 = Q * Q
    t1 = t0 * t0
    t1l = t1 * C1
    t2 = t1l * t1l
    e = t2 * t2
    mp = e + Src1
    spec_step = Spec(body=(mp < One) * mp, reference=_ref_step)

    ops = []
    for name, spec in (("EIF_POLY_ANT", spec_poly), ("EIF_STEP_ANT", spec_step)):
        existing = [o for o in dve_ops.OPS if o.name == name]
        if existing:
            ops.append(existing[0])
            continue
        row = dve_ops._CUSTOM_DVE_ROW_BASE + len(dve_ops.OPS)
        shas = {}
        for ver in ("v3", "v4"):
            try:
                uops = lower(spec, ver=ver)
            except Exception:
                continue
            shas[ver] = DveOpSpec(
                name=name, opcode=row, uops=uops, rd1_en=_has_src1(spec)
            ).sha(ver)
        op = dve_ops.DveOp(name, spec, subdim=False, uops_sha=shas)
        dve_ops.OPS.append(op)
        dve_ops.CUSTOM_DVE_SPECS[name] = spec
        dve_ops._SUB_OPCODE_FOR_NAME[name] = row
        assert row < 0x20
        ops.append(op)
    _registered = tuple(ops)
    return _registered


def _build(reps=1, tc=TC, xbufs=2, sbufs=2, chains=2):
    """chains=1: one 128-col chain, every DVE instr depends on the previous
    (pipeline drains between them).  chains=2: two independent 64-col chains
    interleaved [1X,1Y,2X,2Y] so every RAW is 2 instructions apart and the
    engine can overlap access latency with the neighbor chain."""
    op_poly, op_step = _register_ops()
    nc = bacc.Bacc("TRN2", debug=False, num_devices=NCORES)
    x_d = nc.declare_dram_parameter("x", [P, T * FD], F32, isOutput=False)
    s_d = nc.declare_dram_parameter("spk", [P, T * FD], F32, isOutput=True)

    gf = FD // chains
    with ExitStack() as ctx:
        tcx = ctx.enter_context(tile.TileContext(nc))
        xpool = ctx.enter_context(tcx.tile_pool(name="xin", bufs=xbufs))
        spool = ctx.enter_context(tcx.tile_pool(name="sout", bufs=sbufs))
        state = ctx.enter_context(tcx.tile_pool(name="state", bufs=1))

        mz = state.tile([P, FD], F32, name="mz", tag="mz")
        dlt = state.tile([P, 1], F32, name="dlt", tag="dlt")
        nc.vector.memset(mz[:], 0.0)
        nc.vector.memset(dlt[:], float(DE))
        q = [state.tile([P, gf], F32, name=f"q{g}", tag=f"q{g}")
             for g in range(chains)]

        prev = [mz[:, g * gf:(g + 1) * gf] for g in range(chains)]
        for _rep in range(reps):
            for ci in range(T // tc):
                xt = xpool.tile([P, tc * FD], F32, name="xt", tag="x")
                nc.sync.dma_start(
                    out=xt[:], in_=x_d[:, ci * tc * FD:(ci + 1) * tc * FD]
                )
                xt3 = xt.rearrange("p (t f) -> p t f", f=FD)
                sp = spool.tile([P, tc * FD], F32, name="sp", tag="s")
                sp3 = sp.rearrange("p (t f) -> p t f", f=FD)

                for k in range(tc):
                    for g in range(chains):
                        # INSTR1: q = quartic(m_prev + 1)
                        nc.vector._custom_dve(
                            op_poly, out=q[g][:], in0=prev[g], in1=dlt[:],
                            s0=float(AL), s1=float(BE), imm2=float(GA),
                        )
                    for g in range(chains):
                        # INSTR2: out = (m'<1)*m', m' = (a4*q)^16 + x
                        nc.vector._custom_dve(
                            op_step, out=sp3[:, k, g * gf:(g + 1) * gf],
                            in0=q[g][:],
                            in1=xt3[:, k:k + 1, g * gf:(g + 1) * gf],
                            s1=float(LAM),
                        )
                        prev[g] = sp3[:, k, g * gf:(g + 1) * gf]
                nc.sync.dma_start(
                    out=s_d[:, ci * tc * FD:(ci + 1) * tc * FD], in_=sp[:]
                )
    nc.compile()
    return nc


def _shard(x):
    """x[B,T,N] -> per-core [P, T*FD] partition-major arrays."""
    maps = []
    for c in range(NCORES):
        xc = x[c * BPC:(c + 1) * BPC]                      # [4, T, 4096]
        xc = np.ascontiguousarray(
            xc.reshape(BPC, T, N // FD, FD).transpose(0, 2, 1, 3)
        ).reshape(P, T * FD)
        maps.append({"x": xc})
    return maps


def _unshard(results):
    out = np.empty((B, T, N), np.float32)
    for c in range(NCORES):
        r = np.ascontiguousarray(np.asarray(results[c]["spk"]))
        r = r.reshape(BPC, N // FD, T, FD).transpose(0, 2, 1, 3).reshape(BPC, T, N)
        # spike fired iff the reset wrote +0.0 (bit-exact test)
        out[c * BPC:(c + 1) * BPC] = (r.view(np.uint32) == 0).astype(np.float32)
    return out


def kernel(x):
    global _built
    x = np.asarray(x, dtype=np.float32)
    assert x.shape == (B, T, N), x.shape
    if _built is None:
        _built = _build()
    res = run_bass_kernel_spmd(_built, _shard(x), list(range(NCORES)))
    return _unshard(res.results)
